# revision 3
# baseline (speedup 1.0000x reference)
"""BlockSparseMLP (MoE top-2, 8 experts) — Trainium2 Bass kernel, balanced.

Sharding: instead of expert-per-core (which pads every core to the most
popular expert's token count), each core processes ALL 8 experts but only
a 1/8 slice of the F dimension (704 of 5632 columns).  Per-core weight
bytes stay exactly one expert-equivalent (F-slicing is traffic-neutral),
while the compute is perfectly balanced: every core does sum(n_e) = 4096
token-equivalents with NO padding.  Per-core partial d outputs (each a
partial sum over its F-slice) are weighted by the combine weights
on-device and summed across cores on the host (the unshard step).

The 704-column slice is 5 full 128-col PE tiles plus a 64-col half tile.
Half tiles are processed at full PE efficiency by pairing experts
(2k, 2k+1):
  phase 1: two col-tiled MMs (tile_position (0,0)/(0,64)) into separate
           PSUM banks run concurrently in different column groups.
  phase 2: the packed activations [ah_e2k ; ah_e2k1] feed two row-tiled
           K=64 MMs (stationary at partitions 0:64 / 64:128) that
           accumulate onto each expert's full-K d psum.

Everything bf16 on the wire and in the PE (fp32 PSUM), host pre-casts.
"""

import os
from itertools import zip_longest

import numpy as np
import ml_dtypes

T, D, F, E, TOPK = 2048, 2048, 5632, 8, 2
P = 128
KD = D // P       # 16 contraction subtiles over D
FS = F // E       # 704 F-columns per core
NFT = FS // P     # 5 full f-tiles
HOFF = NFT * P    # 640: half-tile column offset inside the slice
NPAIR = E // 2

BF16 = ml_dtypes.bfloat16

_COMPILED = {}     # tuple(m_e) -> nc
LAST_RESULT = None


def _to_bf16(a):
    """fp32 ndarray -> bf16 (RNE), vectorized."""
    u = np.ascontiguousarray(a, dtype=np.float32).view(np.uint32)
    r = ((u + np.uint32(0x7FFF) + ((u >> np.uint32(16)) & np.uint32(1)))
         >> np.uint32(16)).astype(np.uint16)
    return r.view(BF16)


def _chunks(m):
    """Token chunks <= 512 (PSUM bank limit), each even."""
    if m <= 512:
        return [m]
    h = ((m // 2 + 1) // 2) * 2
    return [h, m - h]


def _build(ms):
    """Build + compile the SPMD Tile program for per-expert token counts
    ``ms`` (tuple of 8 even ints)."""
    import concourse.bass as bass  # noqa: F401
    import concourse.mybir as mybir
    import concourse.tile as tile
    from concourse import bacc

    f32 = mybir.dt.float32
    bf16 = mybir.dt.bfloat16
    mult = mybir.AluOpType.mult
    silu = mybir.ActivationFunctionType.Silu

    mpad = max(ms)
    echunks = [[(sum(_chunks(m)[:i]), c) for i, c in enumerate(_chunks(m))]
               for m in ms]

    nc = bacc.Bacc("TRN2", target_bir_lowering=False, debug=False,
                   enable_asserts=False, num_devices=E)

    wg_d = nc.dram_tensor("wg", [E, P, KD, FS], bf16,
                          kind="ExternalInput").ap()
    wu_d = nc.dram_tensor("wu", [E, P, KD, FS], bf16,
                          kind="ExternalInput").ap()
    wd_d = nc.dram_tensor("wd", [E, P, NFT, D], bf16,
                          kind="ExternalInput").ap()
    wdh_d = nc.dram_tensor("wdh", [NPAIR, P, D], bf16,
                           kind="ExternalInput").ap()
    xt_d = [nc.dram_tensor(f"xt{e}", [P, KD, ms[e]], bf16,
                           kind="ExternalInput").ap() for e in range(E)]
    wr_d = [nc.dram_tensor(f"wr{e}", [P, ms[e]], f32,
                           kind="ExternalInput").ap() for e in range(E)]
    out_d = [nc.dram_tensor(f"out{e}", [KD, P, ms[e]], f32,
                            kind="ExternalOutput").ap() for e in range(E)]
    scr_d = nc.dram_tensor("scr", [P, 512], f32).ap()   # warm-up sink

    with tile.TileContext(nc) as tc:
        with (
            tc.tile_pool(name="resident", bufs=1) as rpool,
            tc.tile_pool(name="xtp", bufs=2) as xtpool,
            tc.tile_pool(name="w1", bufs=3) as w1pool,
            tc.tile_pool(name="whp", bufs=3) as whpool,
            tc.tile_pool(name="wdp", bufs=2) as wdpool,
            tc.tile_pool(name="wdhp", bufs=2) as wdhpool,
            tc.tile_pool(name="ap", bufs=3) as apool,
            tc.tile_pool(name="ahp", bufs=2) as ahpool,
            tc.tile_pool(name="wrp", bufs=3) as wrpool,
            tc.tile_pool(name="outp", bufs=2) as outpool,
            tc.tile_pool(name="psum", bufs=2, space="PSUM") as ppool,
        ):
            # Warm-up: open the PE HAM clock gate while first DMAs fly.
            warm = rpool.tile([P, 512], bf16)
            nc.vector.memset(warm[:], 0.0)
            wps = ppool.tile([P, 512], f32, tag="pg", name="warm_ps")
            for i in range(20):
                nc.tensor.matmul(wps[:], warm[:, :P], warm[:],
                                 start=(i == 0), stop=(i == 19))
            wout = rpool.tile([P, 512], f32)
            nc.vector.tensor_copy(out=wout[:], in_=wps[:])
            nc.sync.dma_start(scr_d[:], wout[:])

            xts, wgs, wus, whs, a_s, ahs, wrs = {}, {}, {}, {}, {}, {}, {}

            def load_expert(e, xt_first):
                """Issue input DMAs for expert e (order = arrival order).

                Main phase-1 weights are split 0:384 / 384:640 so each
                piece frees right after its full-tile MMs; the half-tile
                columns 640:704 live in small separate tiles that stay
                alive until the pair-halves pass (keeps the big tiles'
                buffer rotation from stalling the HWDGE ring).
                """
                m = ms[e]
                xt = xtpool.tile([P, KD, mpad], bf16, tag="xt",
                                 name=f"xt_{e}")
                wgh0 = w1pool.tile([P, KD, 384], bf16, tag="wg",
                                   name=f"wg_{e}_h0")
                wgh1 = w1pool.tile([P, KD, 384], bf16, tag="wg",
                                   name=f"wg_{e}_h1")
                wuh0 = w1pool.tile([P, KD, 384], bf16, tag="wu",
                                   name=f"wu_{e}_h0")
                wuh1 = w1pool.tile([P, KD, 384], bf16, tag="wu",
                                   name=f"wu_{e}_h1")
                wgh = whpool.tile([P, KD, 64], bf16, tag="wgh",
                                  name=f"wgh_{e}")
                wuh = whpool.tile([P, KD, 64], bf16, tag="wuh",
                                  name=f"wuh_{e}")
                wr = wrpool.tile([P, mpad], f32, tag="wr", name=f"wr_{e}")
                if xt_first:
                    nc.sync.dma_start(xt[:, :KD // 2, :m],
                                      xt_d[e][:, :KD // 2, :])
                    nc.sync.dma_start(wgh0[:], wg_d[e][:, :, 0:384])
                    nc.sync.dma_start(xt[:, KD // 2:, :m],
                                      xt_d[e][:, KD // 2:, :])
                    nc.sync.dma_start(wuh0[:], wu_d[e][:, :, 0:384])
                    nc.sync.dma_start(wgh1[:, :, :256],
                                      wg_d[e][:, :, 384:640])
                    nc.sync.dma_start(wuh1[:, :, :256],
                                      wu_d[e][:, :, 384:640])
                else:
                    nc.sync.dma_start(wgh0[:], wg_d[e][:, :, 0:384])
                    nc.sync.dma_start(wuh0[:], wu_d[e][:, :, 0:384])
                    nc.sync.dma_start(wgh1[:, :, :256],
                                      wg_d[e][:, :, 384:640])
                    nc.sync.dma_start(wuh1[:, :, :256],
                                      wu_d[e][:, :, 384:640])
                    nc.sync.dma_start(xt[:, :KD // 2, :m],
                                      xt_d[e][:, :KD // 2, :])
                    nc.sync.dma_start(xt[:, KD // 2:, :m],
                                      xt_d[e][:, KD // 2:, :])
                nc.sync.dma_start(wgh[:], wg_d[e][:, :, HOFF:FS])
                nc.sync.dma_start(wuh[:], wu_d[e][:, :, HOFF:FS])
                nc.sync.dma_start(wr[:, :m], wr_d[e])
                xts[e] = xt
                wgs[e] = (wgh0, wgh1)
                wus[e] = (wuh0, wuh1)
                whs[e] = (wgh, wuh)
                wrs[e] = wr

            def w1sl(halves, ko, f0, f1):
                """Slice [f0:f1) of the 0:640 split phase-1 weight tiles."""
                if f1 <= 384:
                    return halves[0][:, ko, f0:f1]
                assert f0 >= 384
                return halves[1][:, ko, f0 - 384:f1 - 384]

            def phase1_full(e):
                m = ms[e]
                at = apool.tile([P, NFT, mpad], bf16, tag="a", name=f"a_{e}")
                a_s[e] = at
                for ft in range(NFT):
                    for (c0, cn) in echunks[e]:
                        pg = ppool.tile([P, 512], f32, tag="pg")
                        pu = ppool.tile([P, 512], f32, tag="pu")
                        for ko in range(KD):
                            nc.tensor.matmul(
                                pg[:, :cn],
                                w1sl(wgs[e], ko, ft * P, (ft + 1) * P),
                                xts[e][:, ko, c0:c0 + cn],
                                start=(ko == 0), stop=(ko == KD - 1))
                        for ko in range(KD):
                            nc.tensor.matmul(
                                pu[:, :cn],
                                w1sl(wus[e], ko, ft * P, (ft + 1) * P),
                                xts[e][:, ko, c0:c0 + cn],
                                start=(ko == 0), stop=(ko == KD - 1))
                        a_sl = at[:, ft, c0:c0 + cn]
                        nc.scalar.activation(a_sl, pg[:, :cn], silu)
                        nc.vector.tensor_tensor(a_sl, a_sl, pu[:, :cn], mult)

            def phase1_halves(pr):
                """Half f-tile (cols 640:704) for pair (2pr, 2pr+1):
                col-tiled concurrent MMs, separate PSUM banks."""
                e0, e1 = 2 * pr, 2 * pr + 1
                ah = ahpool.tile([P, mpad], bf16, tag="ah", name=f"ah_{pr}")
                ahs[pr] = ah
                for ca, cb in zip_longest(echunks[e0], echunks[e1]):
                    todo = []
                    if ca is not None:
                        todo.append((e0, 0, ca))
                    if cb is not None:
                        todo.append((e1, 64, cb))
                    pgs = {e: ppool.tile([P, 512], f32, tag="pg",
                                         name=f"pgh_{e}_{c0}")
                           for e, lo, (c0, cn) in todo}
                    pus = {e: ppool.tile([P, 512], f32, tag="pu",
                                         name=f"puh_{e}_{c0}")
                           for e, lo, (c0, cn) in todo}
                    for ko in range(KD):
                        for e, lo, (c0, cn) in todo:
                            nc.tensor.matmul(
                                pgs[e][lo:lo + 64, :cn],
                                whs[e][0][:, ko, :],
                                xts[e][:, ko, c0:c0 + cn],
                                start=(ko == 0), stop=(ko == KD - 1),
                                tile_position=(0, lo),
                                skip_group_check=True)
                    for ko in range(KD):
                        for e, lo, (c0, cn) in todo:
                            nc.tensor.matmul(
                                pus[e][lo:lo + 64, :cn],
                                whs[e][1][:, ko, :],
                                xts[e][:, ko, c0:c0 + cn],
                                start=(ko == 0), stop=(ko == KD - 1),
                                tile_position=(0, lo),
                                skip_group_check=True)
                    for e, lo, (c0, cn) in todo:
                        a_sl = ah[lo:lo + 64, c0:c0 + cn]
                        nc.scalar.activation(a_sl, pgs[e][lo:lo + 64, :cn],
                                             silu)
                        nc.vector.tensor_tensor(a_sl, a_sl,
                                                pus[e][lo:lo + 64, :cn], mult)

            def phase2(pr, wds, wdh):
                e0, e1 = 2 * pr, 2 * pr + 1
                for dt in range(KD):
                    dsl = slice(dt * P, (dt + 1) * P)
                    pds = {}
                    for e, lo, ptag in ((e0, 0, "pd0"), (e1, 64, "pd1")):
                        for ci, (c0, cn) in enumerate(echunks[e]):
                            pd = ppool.tile([P, 512], f32, tag=ptag,
                                            name=f"pd_{e}_{dt}_{ci}")
                            pds[(e, ci)] = pd
                            for fk in range(NFT):
                                nc.tensor.matmul(
                                    pd[:, :cn], wds[e][:, fk, dsl],
                                    a_s[e][:, fk, c0:c0 + cn],
                                    start=(fk == 0), stop=False)
                    # half contractions, row-tiled, emitted adjacently
                    for ca, cb in zip_longest(echunks[e0], echunks[e1]):
                        for e, lo, ch in ((e0, 0, ca), (e1, 64, cb)):
                            if ch is None:
                                continue
                            c0, cn = ch
                            ci = echunks[e].index(ch)
                            nc.tensor.matmul(
                                pds[(e, ci)][:, :cn],
                                wdh[lo:lo + 64, dsl],
                                ahs[pr][lo:lo + 64, c0:c0 + cn],
                                start=False, stop=True,
                                skip_group_check=True)
                    for e, lo, _pt in ((e0, 0, "pd0"), (e1, 64, "pd1")):
                        m = ms[e]
                        ot = outpool.tile([P, mpad], f32, tag="ot",
                                          name=f"ot_{e}_{dt}")
                        for ci, (c0, cn) in enumerate(echunks[e]):
                            nc.vector.tensor_tensor(
                                ot[:, c0:c0 + cn], pds[(e, ci)][:, :cn],
                                wrs[e][:, c0:c0 + cn], mult)
                        nc.scalar.dma_start(out_d[e][dt][:, :m], ot[:, :m])

            load_expert(0, xt_first=True)
            load_expert(1, xt_first=True)
            for pr in range(NPAIR):
                e0, e1 = 2 * pr, 2 * pr + 1
                # phase-2 weights for this pair (consumed ~a full phase-1
                # later; split halves so buffer rotation stays fluid)
                wds = {}
                for e in (e0, e1):
                    wdt = wdpool.tile([P, NFT, D], bf16, tag="wd",
                                      name=f"wd_{e}")
                    nc.sync.dma_start(wdt[:, :, 0:1024], wd_d[e][:, :, 0:1024])
                    nc.sync.dma_start(wdt[:, :, 1024:D],
                                      wd_d[e][:, :, 1024:D])
                    wds[e] = wdt
                wdh = wdhpool.tile([P, D], bf16, tag="wdh", name=f"wdh_{pr}")
                nc.sync.dma_start(wdh[:], wdh_d[pr])

                phase1_full(e0)
                if e1 + 1 < E:
                    load_expert(e1 + 1, xt_first=False)
                phase1_full(e1)
                if e1 + 2 < E:
                    load_expert(e1 + 2, xt_first=False)
                phase1_halves(pr)
                phase2(pr, wds, wdh)
        # pools close

    nc.compile()
    return nc


def kernel(x, gate_tensor, Wg, Wu, Wd):
    global LAST_RESULT
    from concourse.bass_interp import get_hw_module
    from concourse.bass_utils import run_bass_kernel_spmd

    x = np.ascontiguousarray(np.asarray(x, dtype=np.float32))
    gate_tensor = np.asarray(gate_tensor, dtype=np.float32)

    # ---- router (replicated; tiny) ----
    logits = x @ gate_tensor
    mx = logits.max(axis=-1, keepdims=True)
    p = np.exp(logits - mx, dtype=np.float32)
    p /= p.sum(axis=-1, keepdims=True)
    topi = np.argsort(-p, axis=-1, kind="stable")[:, :TOPK]
    topw = np.take_along_axis(p, topi, axis=-1)
    topw = topw / (topw.sum(axis=-1, keepdims=True) + 1e-20)

    idx, wts = [], []
    for e in range(E):
        sel = (topi == e)
        idx.append(np.nonzero(sel.any(axis=-1))[0])
        wts.append(topw[sel].astype(np.float32))
    ns = [len(t) for t in idx]
    ms = tuple(max(2, n + (n & 1)) for n in ns)

    if ms not in _COMPILED:
        _COMPILED[ms] = _build(ms)
    nc = _COMPILED[ms]

    xb = _to_bf16(x)
    Wg_b = [_to_bf16(Wg[e]) for e in range(E)]
    Wu_b = [_to_bf16(Wu[e]) for e in range(E)]
    Wd_b = [_to_bf16(Wd[e]) for e in range(E)]

    # shared per-expert token tensors (same arrays for every core)
    xts, wrs = [], []
    for e in range(E):
        n, m = ns[e], ms[e]
        xt = np.zeros((P, KD, m), dtype=BF16)
        xt[:, :, :n] = xb[idx[e]].T.reshape(KD, P, n).transpose(1, 0, 2)
        wr = np.zeros((P, m), dtype=np.float32)
        wr[:, :n] = wts[e][None, :]
        xts.append(xt)
        wrs.append(wr)

    in_maps = []
    for c in range(E):
        c0 = c * FS
        wg = np.stack([Wg_b[e][:, c0:c0 + FS].reshape(KD, P, FS)
                       .transpose(1, 0, 2) for e in range(E)])
        wu = np.stack([Wu_b[e][:, c0:c0 + FS].reshape(KD, P, FS)
                       .transpose(1, 0, 2) for e in range(E)])
        wd = np.stack([Wd_b[e][c0:c0 + HOFF].reshape(NFT, P, D)
                       .transpose(1, 0, 2) for e in range(E)])
        wdh = np.stack([np.concatenate(
            [Wd_b[2 * pr][c0 + HOFF:c0 + FS],
             Wd_b[2 * pr + 1][c0 + HOFF:c0 + FS]], axis=0)
            for pr in range(NPAIR)])
        im = {"wg": np.ascontiguousarray(wg),
              "wu": np.ascontiguousarray(wu),
              "wd": np.ascontiguousarray(wd),
              "wdh": np.ascontiguousarray(wdh)}
        for e in range(E):
            im[f"xt{e}"] = xts[e]
            im[f"wr{e}"] = wrs[e]
        in_maps.append(im)

    trace = bool(int(os.environ.get("KERNEL_TRACE", "0")))
    old_m = nc.m
    nc.m = get_hw_module(nc.m)
    try:
        try:
            res = run_bass_kernel_spmd(nc, in_maps, core_ids=list(range(E)),
                                       trace=trace)
        except (ImportError, ModuleNotFoundError):
            os.environ["BASS_NEVER_TRACE"] = "1"
            res = run_bass_kernel_spmd(nc, in_maps, core_ids=list(range(E)),
                                       trace=False)
    finally:
        nc.m = old_m
    LAST_RESULT = res

    # ---- combine: sum per-core F-slice partials, scatter into out ----
    out = np.zeros((T, D), dtype=np.float32)
    for e in range(E):
        n = ns[e]
        acc = res.results[0][f"out{e}"].astype(np.float64)
        for c in range(1, E):
            acc += res.results[c][f"out{e}"]
        out[idx[e]] += acc.reshape(D, ms[e])[:, :n].T.astype(np.float32)
    return out


# revision 4
# speedup vs baseline: 1.0277x; 1.0277x over previous
"""BlockSparseMLP (MoE top-2, 8 experts) — Trainium2 Bass kernel, balanced.

Sharding: instead of expert-per-core (which pads every core to the most
popular expert's token count), each core processes ALL 8 experts but only
a 1/8 slice of the F dimension (704 of 5632 columns).  Per-core weight
bytes stay exactly one expert-equivalent (F-slicing is traffic-neutral),
while the compute is perfectly balanced: every core does sum(n_e) = 4096
token-equivalents with NO padding.  Per-core partial d outputs (each a
partial sum over its F-slice) are weighted by the combine weights
on-device and summed across cores on the host (the unshard step).

The 704-column slice is 5 full 128-col PE tiles plus a 64-col half tile.
Half tiles are processed at full PE efficiency by pairing experts
(2k, 2k+1):
  phase 1: two col-tiled MMs (tile_position (0,0)/(0,64)) into separate
           PSUM banks run concurrently in different column groups.
  phase 2: the packed activations [ah_e2k ; ah_e2k1] feed two row-tiled
           K=64 MMs (stationary at partitions 0:64 / 64:128) that
           accumulate onto each expert's full-K d psum.

Everything bf16 on the wire and in the PE (fp32 PSUM), host pre-casts.
"""

import os
from itertools import zip_longest

import numpy as np
import ml_dtypes

T, D, F, E, TOPK = 2048, 2048, 5632, 8, 2
P = 128
KD = D // P       # 16 contraction subtiles over D
FS = F // E       # 704 F-columns per core
NFT = FS // P     # 5 full f-tiles
HOFF = NFT * P    # 640: half-tile column offset inside the slice
NPAIR = E // 2

BF16 = ml_dtypes.bfloat16

_COMPILED = {}     # tuple(m_e) -> nc
LAST_RESULT = None


def _to_bf16(a):
    """fp32 ndarray -> bf16 (RNE), vectorized."""
    u = np.ascontiguousarray(a, dtype=np.float32).view(np.uint32)
    r = ((u + np.uint32(0x7FFF) + ((u >> np.uint32(16)) & np.uint32(1)))
         >> np.uint32(16)).astype(np.uint16)
    return r.view(BF16)


def _chunks(m):
    """Token chunks <= 512 (PSUM bank limit), each even."""
    if m <= 512:
        return [m]
    h = ((m // 2 + 1) // 2) * 2
    return [h, m - h]


def _build(ms):
    """Build + compile the SPMD Tile program for per-expert token counts
    ``ms`` (tuple of 8 even ints)."""
    import concourse.bass as bass  # noqa: F401
    import concourse.mybir as mybir
    import concourse.tile as tile
    from concourse import bacc

    f32 = mybir.dt.float32
    bf16 = mybir.dt.bfloat16
    mult = mybir.AluOpType.mult
    silu = mybir.ActivationFunctionType.Silu

    mpad = max(ms)
    echunks = [[(sum(_chunks(m)[:i]), c) for i, c in enumerate(_chunks(m))]
               for m in ms]

    nc = bacc.Bacc("TRN2", target_bir_lowering=False, debug=False,
                   enable_asserts=False, num_devices=E)

    wg_d = nc.dram_tensor("wg", [E, P, KD, FS], bf16,
                          kind="ExternalInput").ap()
    wu_d = nc.dram_tensor("wu", [E, P, KD, FS], bf16,
                          kind="ExternalInput").ap()
    wd_d = nc.dram_tensor("wd", [E, P, NFT, D], bf16,
                          kind="ExternalInput").ap()
    wdh_d = nc.dram_tensor("wdh", [NPAIR, P, D], bf16,
                           kind="ExternalInput").ap()
    xt_d = [nc.dram_tensor(f"xt{e}", [P, KD, ms[e]], bf16,
                           kind="ExternalInput").ap() for e in range(E)]
    wr_d = [nc.dram_tensor(f"wr{e}", [P, ms[e]], f32,
                           kind="ExternalInput").ap() for e in range(E)]
    out_d = [nc.dram_tensor(f"out{e}", [KD, P, ms[e]], f32,
                            kind="ExternalOutput").ap() for e in range(E)]
    scr_d = nc.dram_tensor("scr", [P, 512], f32).ap()   # warm-up sink

    with tile.TileContext(nc) as tc:
        with (
            tc.tile_pool(name="resident", bufs=1) as rpool,
            tc.tile_pool(name="xtp", bufs=2) as xtpool,
            tc.tile_pool(name="w1", bufs=3) as w1pool,
            tc.tile_pool(name="whp", bufs=3) as whpool,
            tc.tile_pool(name="wdp", bufs=2) as wdpool,
            tc.tile_pool(name="wdhp", bufs=1) as wdhpool,
            tc.tile_pool(name="ap", bufs=3) as apool,
            tc.tile_pool(name="ahp", bufs=2) as ahpool,
            tc.tile_pool(name="wrp", bufs=3) as wrpool,
            tc.tile_pool(name="outp", bufs=4) as outpool,
            tc.tile_pool(name="psum", bufs=2, space="PSUM") as ppool,
        ):
            # Warm-up: open the PE HAM clock gate while first DMAs fly.
            warm = rpool.tile([P, 512], bf16)
            nc.vector.memset(warm[:], 0.0)
            wps = ppool.tile([P, 512], f32, tag="pg", name="warm_ps")
            NWARM = 28
            for i in range(NWARM):
                nc.tensor.matmul(wps[:], warm[:, :P], warm[:],
                                 start=(i == 0), stop=(i == NWARM - 1))
            wout = rpool.tile([P, P], f32)
            nc.vector.tensor_copy(out=wout[:], in_=wps[:, :P])
            nc.sync.dma_start(scr_d[:, :P], wout[:])

            xts, wgs, wus, whs, a_s, ahs, wrs = {}, {}, {}, {}, {}, {}, {}

            def load_expert(e, xt_first):
                """Issue input DMAs for expert e (order = arrival order).

                Main phase-1 weights are split 0:384 / 384:640 so each
                piece frees right after its full-tile MMs; the half-tile
                columns 640:704 live in small separate tiles that stay
                alive until the pair-halves pass (keeps the big tiles'
                buffer rotation from stalling the HWDGE ring).
                """
                m = ms[e]
                xt = xtpool.tile([P, KD, mpad], bf16, tag="xt",
                                 name=f"xt_{e}")
                wgh0 = w1pool.tile([P, KD, 384], bf16, tag="wg",
                                   name=f"wg_{e}_h0")
                wgh1 = w1pool.tile([P, KD, 384], bf16, tag="wg",
                                   name=f"wg_{e}_h1")
                wuh0 = w1pool.tile([P, KD, 384], bf16, tag="wu",
                                   name=f"wu_{e}_h0")
                wuh1 = w1pool.tile([P, KD, 384], bf16, tag="wu",
                                   name=f"wu_{e}_h1")
                wgh = whpool.tile([P, KD, 64], bf16, tag="wgh",
                                  name=f"wgh_{e}")
                wuh = whpool.tile([P, KD, 64], bf16, tag="wuh",
                                  name=f"wuh_{e}")
                wr = wrpool.tile([P, mpad], f32, tag="wr", name=f"wr_{e}")
                if xt_first:
                    nc.sync.dma_start(xt[:, :KD // 2, :m],
                                      xt_d[e][:, :KD // 2, :])
                    nc.sync.dma_start(wgh0[:], wg_d[e][:, :, 0:384])
                    nc.sync.dma_start(xt[:, KD // 2:, :m],
                                      xt_d[e][:, KD // 2:, :])
                    nc.sync.dma_start(wuh0[:], wu_d[e][:, :, 0:384])
                    nc.sync.dma_start(wgh1[:, :, :256],
                                      wg_d[e][:, :, 384:640])
                    nc.sync.dma_start(wuh1[:, :, :256],
                                      wu_d[e][:, :, 384:640])
                else:
                    nc.sync.dma_start(wgh0[:], wg_d[e][:, :, 0:384])
                    nc.sync.dma_start(wuh0[:], wu_d[e][:, :, 0:384])
                    nc.sync.dma_start(wgh1[:, :, :256],
                                      wg_d[e][:, :, 384:640])
                    nc.sync.dma_start(wuh1[:, :, :256],
                                      wu_d[e][:, :, 384:640])
                    nc.sync.dma_start(xt[:, :KD // 2, :m],
                                      xt_d[e][:, :KD // 2, :])
                    nc.sync.dma_start(xt[:, KD // 2:, :m],
                                      xt_d[e][:, KD // 2:, :])
                nc.sync.dma_start(wgh[:], wg_d[e][:, :, HOFF:FS])
                nc.sync.dma_start(wuh[:], wu_d[e][:, :, HOFF:FS])
                nc.sync.dma_start(wr[:, :m], wr_d[e])
                xts[e] = xt
                wgs[e] = (wgh0, wgh1)
                wus[e] = (wuh0, wuh1)
                whs[e] = (wgh, wuh)
                wrs[e] = wr

            def w1sl(halves, ko, f0, f1):
                """Slice [f0:f1) of the 0:640 split phase-1 weight tiles."""
                if f1 <= 384:
                    return halves[0][:, ko, f0:f1]
                assert f0 >= 384
                return halves[1][:, ko, f0 - 384:f1 - 384]

            def phase1_full(e):
                m = ms[e]
                at = apool.tile([P, NFT, mpad], bf16, tag="a", name=f"a_{e}")
                a_s[e] = at
                for ft in range(NFT):
                    for (c0, cn) in echunks[e]:
                        pg = ppool.tile([P, 512], f32, tag="pg")
                        pu = ppool.tile([P, 512], f32, tag="pu")
                        for ko in range(KD):
                            nc.tensor.matmul(
                                pg[:, :cn],
                                w1sl(wgs[e], ko, ft * P, (ft + 1) * P),
                                xts[e][:, ko, c0:c0 + cn],
                                start=(ko == 0), stop=(ko == KD - 1))
                        for ko in range(KD):
                            nc.tensor.matmul(
                                pu[:, :cn],
                                w1sl(wus[e], ko, ft * P, (ft + 1) * P),
                                xts[e][:, ko, c0:c0 + cn],
                                start=(ko == 0), stop=(ko == KD - 1))
                        a_sl = at[:, ft, c0:c0 + cn]
                        nc.scalar.activation(a_sl, pg[:, :cn], silu)
                        nc.vector.tensor_tensor(a_sl, a_sl, pu[:, :cn], mult)

            def phase1_halves(pr):
                """Half f-tile (cols 640:704) for pair (2pr, 2pr+1):
                col-tiled concurrent MMs, separate PSUM banks."""
                e0, e1 = 2 * pr, 2 * pr + 1
                ah = ahpool.tile([P, mpad], bf16, tag="ah", name=f"ah_{pr}")
                ahs[pr] = ah
                for ca, cb in zip_longest(echunks[e0], echunks[e1]):
                    todo = []
                    if ca is not None:
                        todo.append((e0, 0, ca))
                    if cb is not None:
                        todo.append((e1, 64, cb))
                    pgs = {e: ppool.tile([P, 512], f32, tag="pg",
                                         name=f"pgh_{e}_{c0}")
                           for e, lo, (c0, cn) in todo}
                    pus = {e: ppool.tile([P, 512], f32, tag="pu",
                                         name=f"puh_{e}_{c0}")
                           for e, lo, (c0, cn) in todo}
                    for ko in range(KD):
                        for e, lo, (c0, cn) in todo:
                            nc.tensor.matmul(
                                pgs[e][lo:lo + 64, :cn],
                                whs[e][0][:, ko, :],
                                xts[e][:, ko, c0:c0 + cn],
                                start=(ko == 0), stop=(ko == KD - 1),
                                tile_position=(0, lo),
                                skip_group_check=True)
                    for ko in range(KD):
                        for e, lo, (c0, cn) in todo:
                            nc.tensor.matmul(
                                pus[e][lo:lo + 64, :cn],
                                whs[e][1][:, ko, :],
                                xts[e][:, ko, c0:c0 + cn],
                                start=(ko == 0), stop=(ko == KD - 1),
                                tile_position=(0, lo),
                                skip_group_check=True)
                    for e, lo, (c0, cn) in todo:
                        a_sl = ah[lo:lo + 64, c0:c0 + cn]
                        nc.scalar.activation(a_sl, pgs[e][lo:lo + 64, :cn],
                                             silu)
                        nc.vector.tensor_tensor(a_sl, a_sl,
                                                pus[e][lo:lo + 64, :cn], mult)

            def phase2(pr, wds, wdh):
                e0, e1 = 2 * pr, 2 * pr + 1
                for dt in range(KD):
                    dsl = slice(dt * P, (dt + 1) * P)
                    pds = {}
                    for e, lo, ptag in ((e0, 0, "pd0"), (e1, 64, "pd1")):
                        for ci, (c0, cn) in enumerate(echunks[e]):
                            pd = ppool.tile([P, 512], f32, tag=ptag,
                                            name=f"pd_{e}_{dt}_{ci}")
                            pds[(e, ci)] = pd
                            for fk in range(NFT):
                                nc.tensor.matmul(
                                    pd[:, :cn], wds[e][:, fk, dsl],
                                    a_s[e][:, fk, c0:c0 + cn],
                                    start=(fk == 0), stop=False)
                    # half contractions, row-tiled, emitted adjacently
                    for ca, cb in zip_longest(echunks[e0], echunks[e1]):
                        for e, lo, ch in ((e0, 0, ca), (e1, 64, cb)):
                            if ch is None:
                                continue
                            c0, cn = ch
                            ci = echunks[e].index(ch)
                            nc.tensor.matmul(
                                pds[(e, ci)][:, :cn],
                                wdh[lo:lo + 64, dsl],
                                ahs[pr][lo:lo + 64, c0:c0 + cn],
                                start=False, stop=True,
                                skip_group_check=True)
                    for e, lo, _pt in ((e0, 0, "pd0"), (e1, 64, "pd1")):
                        m = ms[e]
                        ot = outpool.tile([P, mpad], f32, tag="ot",
                                          name=f"ot_{e}_{dt}")
                        for ci, (c0, cn) in enumerate(echunks[e]):
                            nc.vector.tensor_tensor(
                                ot[:, c0:c0 + cn], pds[(e, ci)][:, :cn],
                                wrs[e][:, c0:c0 + cn], mult)
                        nc.scalar.dma_start(out_d[e][dt][:, :m], ot[:, :m])

            load_expert(0, xt_first=True)
            load_expert(1, xt_first=True)
            for pr in range(NPAIR):
                e0, e1 = 2 * pr, 2 * pr + 1
                # phase-2 weights for this pair (consumed ~a full phase-1
                # later; split halves so buffer rotation stays fluid)
                wds = {}
                for e in (e0, e1):
                    wdt = wdpool.tile([P, NFT, D], bf16, tag="wd",
                                      name=f"wd_{e}")
                    nc.sync.dma_start(wdt[:, :, 0:1024], wd_d[e][:, :, 0:1024])
                    nc.sync.dma_start(wdt[:, :, 1024:D],
                                      wd_d[e][:, :, 1024:D])
                    wds[e] = wdt
                wdh = wdhpool.tile([P, D], bf16, tag="wdh", name=f"wdh_{pr}")
                nc.sync.dma_start(wdh[:], wdh_d[pr])

                phase1_full(e0)
                if e1 + 1 < E:
                    load_expert(e1 + 1, xt_first=False)
                phase1_full(e1)
                phase1_halves(pr)
                # after the halves, this pair's xt/half-weight tiles are
                # free — emitting the next load here keeps the HWDGE ring
                # from stalling on their buffer rotation mid-pair
                if e1 + 2 < E:
                    load_expert(e1 + 2, xt_first=False)
                phase2(pr, wds, wdh)
        # pools close

    nc.compile()
    return nc


def kernel(x, gate_tensor, Wg, Wu, Wd):
    global LAST_RESULT
    from concourse.bass_interp import get_hw_module
    from concourse.bass_utils import run_bass_kernel_spmd

    x = np.ascontiguousarray(np.asarray(x, dtype=np.float32))
    gate_tensor = np.asarray(gate_tensor, dtype=np.float32)

    # ---- router (replicated; tiny) ----
    logits = x @ gate_tensor
    mx = logits.max(axis=-1, keepdims=True)
    p = np.exp(logits - mx, dtype=np.float32)
    p /= p.sum(axis=-1, keepdims=True)
    topi = np.argsort(-p, axis=-1, kind="stable")[:, :TOPK]
    topw = np.take_along_axis(p, topi, axis=-1)
    topw = topw / (topw.sum(axis=-1, keepdims=True) + 1e-20)

    idx, wts = [], []
    for e in range(E):
        sel = (topi == e)
        idx.append(np.nonzero(sel.any(axis=-1))[0])
        wts.append(topw[sel].astype(np.float32))
    ns = [len(t) for t in idx]
    ms = tuple(max(2, n + (n & 1)) for n in ns)

    if ms not in _COMPILED:
        _COMPILED[ms] = _build(ms)
    nc = _COMPILED[ms]

    xb = _to_bf16(x)
    Wg_b = [_to_bf16(Wg[e]) for e in range(E)]
    Wu_b = [_to_bf16(Wu[e]) for e in range(E)]
    Wd_b = [_to_bf16(Wd[e]) for e in range(E)]

    # shared per-expert token tensors (same arrays for every core)
    xts, wrs = [], []
    for e in range(E):
        n, m = ns[e], ms[e]
        xt = np.zeros((P, KD, m), dtype=BF16)
        xt[:, :, :n] = xb[idx[e]].T.reshape(KD, P, n).transpose(1, 0, 2)
        wr = np.zeros((P, m), dtype=np.float32)
        wr[:, :n] = wts[e][None, :]
        xts.append(xt)
        wrs.append(wr)

    in_maps = []
    for c in range(E):
        c0 = c * FS
        wg = np.stack([Wg_b[e][:, c0:c0 + FS].reshape(KD, P, FS)
                       .transpose(1, 0, 2) for e in range(E)])
        wu = np.stack([Wu_b[e][:, c0:c0 + FS].reshape(KD, P, FS)
                       .transpose(1, 0, 2) for e in range(E)])
        wd = np.stack([Wd_b[e][c0:c0 + HOFF].reshape(NFT, P, D)
                       .transpose(1, 0, 2) for e in range(E)])
        wdh = np.stack([np.concatenate(
            [Wd_b[2 * pr][c0 + HOFF:c0 + FS],
             Wd_b[2 * pr + 1][c0 + HOFF:c0 + FS]], axis=0)
            for pr in range(NPAIR)])
        im = {"wg": np.ascontiguousarray(wg),
              "wu": np.ascontiguousarray(wu),
              "wd": np.ascontiguousarray(wd),
              "wdh": np.ascontiguousarray(wdh)}
        for e in range(E):
            im[f"xt{e}"] = xts[e]
            im[f"wr{e}"] = wrs[e]
        in_maps.append(im)

    trace = bool(int(os.environ.get("KERNEL_TRACE", "0")))
    old_m = nc.m
    nc.m = get_hw_module(nc.m)
    try:
        try:
            res = run_bass_kernel_spmd(nc, in_maps, core_ids=list(range(E)),
                                       trace=trace)
        except (ImportError, ModuleNotFoundError):
            os.environ["BASS_NEVER_TRACE"] = "1"
            res = run_bass_kernel_spmd(nc, in_maps, core_ids=list(range(E)),
                                       trace=False)
    finally:
        nc.m = old_m
    LAST_RESULT = res

    # ---- combine: sum per-core F-slice partials, scatter into out ----
    out = np.zeros((T, D), dtype=np.float32)
    for e in range(E):
        n = ns[e]
        acc = res.results[0][f"out{e}"].astype(np.float64)
        for c in range(1, E):
            acc += res.results[c][f"out{e}"]
        out[idx[e]] += acc.reshape(D, ms[e])[:, :n].T.astype(np.float32)
    return out


# revision 5
# speedup vs baseline: 1.0783x; 1.0493x over previous
"""BlockSparseMLP (MoE top-2, 8 experts) — Trainium2 Bass kernel, balanced.

Sharding: instead of expert-per-core (which pads every core to the most
popular expert's token count), each core processes ALL 8 experts but only
a 1/8 slice of the F dimension (704 of 5632 columns).  Per-core weight
bytes stay exactly one expert-equivalent (F-slicing is traffic-neutral),
while the compute is perfectly balanced: every core does sum(n_e) = 4096
token-equivalents with NO padding.  Per-core partial d outputs (each a
partial sum over its F-slice) are weighted by the combine weights
on-device and summed across cores on the host (the unshard step).

The 704-column slice is 5 full 128-col PE tiles plus a 64-col half tile.
Half tiles are processed at full PE efficiency by pairing experts
(2k, 2k+1):
  phase 1: two col-tiled MMs (tile_position (0,0)/(0,64)) into separate
           PSUM banks run concurrently in different column groups.
  phase 2: the packed activations [ah_e2k ; ah_e2k1] feed two row-tiled
           K=64 MMs (stationary at partitions 0:64 / 64:128) that
           accumulate onto each expert's full-K d psum.

Everything bf16 on the wire and in the PE (fp32 PSUM), host pre-casts.
"""

import os
from itertools import zip_longest

import numpy as np
import ml_dtypes

T, D, F, E, TOPK = 2048, 2048, 5632, 8, 2
P = 128
KD = D // P       # 16 contraction subtiles over D
FS = F // E       # 704 F-columns per core
NFT = FS // P     # 5 full f-tiles
HOFF = NFT * P    # 640: half-tile column offset inside the slice
NPAIR = E // 2

BF16 = ml_dtypes.bfloat16

_COMPILED = {}     # tuple(m_e) -> nc
LAST_RESULT = None


def _to_bf16(a):
    """fp32 ndarray -> bf16 (RNE), vectorized."""
    u = np.ascontiguousarray(a, dtype=np.float32).view(np.uint32)
    r = ((u + np.uint32(0x7FFF) + ((u >> np.uint32(16)) & np.uint32(1)))
         >> np.uint32(16)).astype(np.uint16)
    return r.view(BF16)


def _chunks(m):
    """Token chunks <= 512 (PSUM bank limit), each even."""
    if m <= 512:
        return [m]
    h = ((m // 2 + 1) // 2) * 2
    return [h, m - h]


def _build(ms):
    """Build + compile the SPMD Tile program for per-expert token counts
    ``ms`` (tuple of 8 even ints)."""
    import concourse.bass as bass  # noqa: F401
    import concourse.mybir as mybir
    import concourse.tile as tile
    from concourse import bacc

    f32 = mybir.dt.float32
    bf16 = mybir.dt.bfloat16
    mult = mybir.AluOpType.mult
    silu = mybir.ActivationFunctionType.Silu

    mpad = max(ms)
    echunks = [[(sum(_chunks(m)[:i]), c) for i, c in enumerate(_chunks(m))]
               for m in ms]

    nc = bacc.Bacc("TRN2", target_bir_lowering=False, debug=False,
                   enable_asserts=False, num_devices=E)

    wg_d = nc.dram_tensor("wg", [E, P, KD, FS], bf16,
                          kind="ExternalInput").ap()
    wu_d = nc.dram_tensor("wu", [E, P, KD, FS], bf16,
                          kind="ExternalInput").ap()
    wd_d = nc.dram_tensor("wd", [E, P, NFT, D], bf16,
                          kind="ExternalInput").ap()
    wdh_d = nc.dram_tensor("wdh", [NPAIR, P, D], bf16,
                           kind="ExternalInput").ap()
    xt_d = [nc.dram_tensor(f"xt{e}", [P, KD, ms[e]], bf16,
                           kind="ExternalInput").ap() for e in range(E)]
    wr_d = [nc.dram_tensor(f"wr{e}", [P, ms[e]], f32,
                           kind="ExternalInput").ap() for e in range(E)]
    out_d = [nc.dram_tensor(f"out{e}", [KD, P, ms[e]], f32,
                            kind="ExternalOutput").ap() for e in range(E)]
    scr_d = nc.dram_tensor("scr", [P, 512], f32).ap()   # warm-up sink

    with tile.TileContext(nc) as tc:
        with (
            tc.tile_pool(name="resident", bufs=1) as rpool,
            tc.tile_pool(name="xtp", bufs=2) as xtpool,
            tc.tile_pool(name="w1", bufs=3) as w1pool,
            tc.tile_pool(name="whp", bufs=3) as whpool,
            tc.tile_pool(name="wdp", bufs=2) as wdpool,
            tc.tile_pool(name="wdhp", bufs=1) as wdhpool,
            tc.tile_pool(name="ap", bufs=3) as apool,
            tc.tile_pool(name="ahp", bufs=2) as ahpool,
            tc.tile_pool(name="wrp", bufs=4) as wrpool,
            tc.tile_pool(name="outp", bufs=4) as outpool,
            tc.tile_pool(name="psum", bufs=2, space="PSUM") as ppool,
        ):
            # Warm-up: open the PE HAM clock gate while first DMAs fly.
            warm = rpool.tile([P, 512], bf16)
            nc.vector.memset(warm[:], 0.0)
            wps = ppool.tile([P, 512], f32, tag="pg", name="warm_ps")
            NWARM = 28
            for i in range(NWARM):
                nc.tensor.matmul(wps[:], warm[:, :P], warm[:],
                                 start=(i == 0), stop=(i == NWARM - 1))
            wout = rpool.tile([P, P], f32)
            nc.vector.tensor_copy(out=wout[:], in_=wps[:, :P])
            nc.sync.dma_start(scr_d[:, :P], wout[:])

            xts, wgs, wus, whs, a_s, ahs, wrs = {}, {}, {}, {}, {}, {}, {}

            def load_expert(e, xt_first):
                """Issue input DMAs for expert e (order = arrival order).

                Main phase-1 weights are split 0:384 / 384:640 so each
                piece frees right after its full-tile MMs; the half-tile
                columns 640:704 live in small separate tiles that stay
                alive until the pair-halves pass (keeps the big tiles'
                buffer rotation from stalling the HWDGE ring).
                """
                m = ms[e]
                xt = xtpool.tile([P, KD, mpad], bf16, tag="xt",
                                 name=f"xt_{e}")
                wgh0 = w1pool.tile([P, KD, 384], bf16, tag="wg",
                                   name=f"wg_{e}_h0")
                wgh1 = w1pool.tile([P, KD, 384], bf16, tag="wg",
                                   name=f"wg_{e}_h1")
                wuh0 = w1pool.tile([P, KD, 384], bf16, tag="wu",
                                   name=f"wu_{e}_h0")
                wuh1 = w1pool.tile([P, KD, 384], bf16, tag="wu",
                                   name=f"wu_{e}_h1")
                wgh = whpool.tile([P, KD, 64], bf16, tag="wgh",
                                  name=f"wgh_{e}")
                wuh = whpool.tile([P, KD, 64], bf16, tag="wuh",
                                  name=f"wuh_{e}")
                wr = wrpool.tile([P, mpad], f32, tag="wr", name=f"wr_{e}")
                if xt_first:
                    nc.sync.dma_start(xt[:, :KD // 2, :m],
                                      xt_d[e][:, :KD // 2, :])
                    nc.sync.dma_start(wgh0[:], wg_d[e][:, :, 0:384])
                    nc.sync.dma_start(xt[:, KD // 2:, :m],
                                      xt_d[e][:, KD // 2:, :])
                    nc.sync.dma_start(wuh0[:], wu_d[e][:, :, 0:384])
                    nc.sync.dma_start(wgh1[:, :, :256],
                                      wg_d[e][:, :, 384:640])
                    nc.sync.dma_start(wuh1[:, :, :256],
                                      wu_d[e][:, :, 384:640])
                else:
                    nc.sync.dma_start(wgh0[:], wg_d[e][:, :, 0:384])
                    nc.sync.dma_start(wuh0[:], wu_d[e][:, :, 0:384])
                    nc.sync.dma_start(wgh1[:, :, :256],
                                      wg_d[e][:, :, 384:640])
                    nc.sync.dma_start(wuh1[:, :, :256],
                                      wu_d[e][:, :, 384:640])
                    nc.sync.dma_start(xt[:, :KD // 2, :m],
                                      xt_d[e][:, :KD // 2, :])
                    nc.sync.dma_start(xt[:, KD // 2:, :m],
                                      xt_d[e][:, KD // 2:, :])
                nc.sync.dma_start(wgh[:], wg_d[e][:, :, HOFF:FS])
                nc.sync.dma_start(wuh[:], wu_d[e][:, :, HOFF:FS])
                nc.gpsimd.dma_start(wr[:, :m], wr_d[e])
                xts[e] = xt
                wgs[e] = (wgh0, wgh1)
                wus[e] = (wuh0, wuh1)
                whs[e] = (wgh, wuh)
                wrs[e] = wr

            def w1sl(halves, ko, f0, f1):
                """Slice [f0:f1) of the 0:640 split phase-1 weight tiles."""
                if f1 <= 384:
                    return halves[0][:, ko, f0:f1]
                assert f0 >= 384
                return halves[1][:, ko, f0 - 384:f1 - 384]

            def phase1_full(e):
                m = ms[e]
                at = apool.tile([P, NFT, mpad], bf16, tag="a", name=f"a_{e}")
                a_s[e] = at
                for ft in range(NFT):
                    for (c0, cn) in echunks[e]:
                        pg = ppool.tile([P, 512], f32, tag="pg")
                        pu = ppool.tile([P, 512], f32, tag="pu")
                        for ko in range(KD):
                            nc.tensor.matmul(
                                pg[:, :cn],
                                w1sl(wgs[e], ko, ft * P, (ft + 1) * P),
                                xts[e][:, ko, c0:c0 + cn],
                                start=(ko == 0), stop=(ko == KD - 1))
                        for ko in range(KD):
                            nc.tensor.matmul(
                                pu[:, :cn],
                                w1sl(wus[e], ko, ft * P, (ft + 1) * P),
                                xts[e][:, ko, c0:c0 + cn],
                                start=(ko == 0), stop=(ko == KD - 1))
                        a_sl = at[:, ft, c0:c0 + cn]
                        nc.scalar.activation(a_sl, pg[:, :cn], silu)
                        nc.vector.tensor_tensor(a_sl, a_sl, pu[:, :cn], mult)

            def phase1_halves(pr):
                """Half f-tile (cols 640:704) for pair (2pr, 2pr+1):
                col-tiled concurrent MMs, separate PSUM banks."""
                e0, e1 = 2 * pr, 2 * pr + 1
                ah = ahpool.tile([P, mpad], bf16, tag="ah", name=f"ah_{pr}")
                ahs[pr] = ah
                for ca, cb in zip_longest(echunks[e0], echunks[e1]):
                    todo = []
                    if ca is not None:
                        todo.append((e0, 0, ca))
                    if cb is not None:
                        todo.append((e1, 64, cb))
                    pgs = {e: ppool.tile([P, 512], f32, tag="pg",
                                         name=f"pgh_{e}_{c0}")
                           for e, lo, (c0, cn) in todo}
                    pus = {e: ppool.tile([P, 512], f32, tag="pu",
                                         name=f"puh_{e}_{c0}")
                           for e, lo, (c0, cn) in todo}
                    for ko in range(KD):
                        for e, lo, (c0, cn) in todo:
                            nc.tensor.matmul(
                                pgs[e][lo:lo + 64, :cn],
                                whs[e][0][:, ko, :],
                                xts[e][:, ko, c0:c0 + cn],
                                start=(ko == 0), stop=(ko == KD - 1),
                                tile_position=(0, lo),
                                skip_group_check=True)
                    for ko in range(KD):
                        for e, lo, (c0, cn) in todo:
                            nc.tensor.matmul(
                                pus[e][lo:lo + 64, :cn],
                                whs[e][1][:, ko, :],
                                xts[e][:, ko, c0:c0 + cn],
                                start=(ko == 0), stop=(ko == KD - 1),
                                tile_position=(0, lo),
                                skip_group_check=True)
                    for e, lo, (c0, cn) in todo:
                        a_sl = ah[lo:lo + 64, c0:c0 + cn]
                        nc.scalar.activation(a_sl, pgs[e][lo:lo + 64, :cn],
                                             silu)
                        nc.vector.tensor_tensor(a_sl, a_sl,
                                                pus[e][lo:lo + 64, :cn], mult)

            def phase2(pr, wds, wdh):
                e0, e1 = 2 * pr, 2 * pr + 1
                for dt in range(KD):
                    dsl = slice(dt * P, (dt + 1) * P)
                    pds = {}
                    for e, lo, ptag in ((e0, 0, "pd0"), (e1, 64, "pd1")):
                        for ci, (c0, cn) in enumerate(echunks[e]):
                            pd = ppool.tile([P, 512], f32, tag=ptag,
                                            name=f"pd_{e}_{dt}_{ci}")
                            pds[(e, ci)] = pd
                            for fk in range(NFT):
                                nc.tensor.matmul(
                                    pd[:, :cn], wds[e][:, fk, dsl],
                                    a_s[e][:, fk, c0:c0 + cn],
                                    start=(fk == 0), stop=False)
                    # half contractions, row-tiled, emitted adjacently
                    for ca, cb in zip_longest(echunks[e0], echunks[e1]):
                        for e, lo, ch in ((e0, 0, ca), (e1, 64, cb)):
                            if ch is None:
                                continue
                            c0, cn = ch
                            ci = echunks[e].index(ch)
                            nc.tensor.matmul(
                                pds[(e, ci)][:, :cn],
                                wdh[lo:lo + 64, dsl],
                                ahs[pr][lo:lo + 64, c0:c0 + cn],
                                start=False, stop=True,
                                skip_group_check=True)
                    for e, lo, _pt in ((e0, 0, "pd0"), (e1, 64, "pd1")):
                        m = ms[e]
                        ot = outpool.tile([P, mpad], f32, tag="ot",
                                          name=f"ot_{e}_{dt}")
                        for ci, (c0, cn) in enumerate(echunks[e]):
                            nc.vector.tensor_tensor(
                                ot[:, c0:c0 + cn], pds[(e, ci)][:, :cn],
                                wrs[e][:, c0:c0 + cn], mult)
                        nc.scalar.dma_start(out_d[e][dt][:, :m], ot[:, :m])

            load_expert(0, xt_first=True)
            load_expert(1, xt_first=True)
            for pr in range(NPAIR):
                e0, e1 = 2 * pr, 2 * pr + 1
                # phase-2 weights for this pair (consumed ~a full phase-1
                # later; split halves so buffer rotation stays fluid)
                # phase-2 weights ride the SWDGE (gpsimd) ring so their
                # buffer-rotation waits never block the phase-1 input
                # stream on the sync ring (head-of-line cascades cost
                # ~19us per pair boundary otherwise)
                wds = {}
                for e in (e0, e1):
                    wdt = wdpool.tile([P, NFT, D], bf16, tag="wd",
                                      name=f"wd_{e}")
                    nc.gpsimd.dma_start(wdt[:, :, 0:1024],
                                        wd_d[e][:, :, 0:1024])
                    nc.gpsimd.dma_start(wdt[:, :, 1024:D],
                                        wd_d[e][:, :, 1024:D])
                    wds[e] = wdt
                wdh = wdhpool.tile([P, D], bf16, tag="wdh", name=f"wdh_{pr}")
                nc.gpsimd.dma_start(wdh[:], wdh_d[pr])

                phase1_full(e0)
                if e1 + 1 < E:
                    load_expert(e1 + 1, xt_first=False)
                phase1_full(e1)
                phase1_halves(pr)
                # after the halves, this pair's xt/half-weight tiles are
                # free — emitting the next load here keeps the HWDGE ring
                # from stalling on their buffer rotation mid-pair
                if e1 + 2 < E:
                    load_expert(e1 + 2, xt_first=False)
                phase2(pr, wds, wdh)
        # pools close

    nc.compile()
    return nc


def kernel(x, gate_tensor, Wg, Wu, Wd):
    global LAST_RESULT
    from concourse.bass_interp import get_hw_module
    from concourse.bass_utils import run_bass_kernel_spmd

    x = np.ascontiguousarray(np.asarray(x, dtype=np.float32))
    gate_tensor = np.asarray(gate_tensor, dtype=np.float32)

    # ---- router (replicated; tiny) ----
    logits = x @ gate_tensor
    mx = logits.max(axis=-1, keepdims=True)
    p = np.exp(logits - mx, dtype=np.float32)
    p /= p.sum(axis=-1, keepdims=True)
    topi = np.argsort(-p, axis=-1, kind="stable")[:, :TOPK]
    topw = np.take_along_axis(p, topi, axis=-1)
    topw = topw / (topw.sum(axis=-1, keepdims=True) + 1e-20)

    idx, wts = [], []
    for e in range(E):
        sel = (topi == e)
        idx.append(np.nonzero(sel.any(axis=-1))[0])
        wts.append(topw[sel].astype(np.float32))
    ns = [len(t) for t in idx]
    ms = tuple(max(2, n + (n & 1)) for n in ns)

    if ms not in _COMPILED:
        _COMPILED[ms] = _build(ms)
    nc = _COMPILED[ms]

    xb = _to_bf16(x)
    Wg_b = [_to_bf16(Wg[e]) for e in range(E)]
    Wu_b = [_to_bf16(Wu[e]) for e in range(E)]
    Wd_b = [_to_bf16(Wd[e]) for e in range(E)]

    # shared per-expert token tensors (same arrays for every core)
    xts, wrs = [], []
    for e in range(E):
        n, m = ns[e], ms[e]
        xt = np.zeros((P, KD, m), dtype=BF16)
        xt[:, :, :n] = xb[idx[e]].T.reshape(KD, P, n).transpose(1, 0, 2)
        wr = np.zeros((P, m), dtype=np.float32)
        wr[:, :n] = wts[e][None, :]
        xts.append(xt)
        wrs.append(wr)

    in_maps = []
    for c in range(E):
        c0 = c * FS
        wg = np.stack([Wg_b[e][:, c0:c0 + FS].reshape(KD, P, FS)
                       .transpose(1, 0, 2) for e in range(E)])
        wu = np.stack([Wu_b[e][:, c0:c0 + FS].reshape(KD, P, FS)
                       .transpose(1, 0, 2) for e in range(E)])
        wd = np.stack([Wd_b[e][c0:c0 + HOFF].reshape(NFT, P, D)
                       .transpose(1, 0, 2) for e in range(E)])
        wdh = np.stack([np.concatenate(
            [Wd_b[2 * pr][c0 + HOFF:c0 + FS],
             Wd_b[2 * pr + 1][c0 + HOFF:c0 + FS]], axis=0)
            for pr in range(NPAIR)])
        im = {"wg": np.ascontiguousarray(wg),
              "wu": np.ascontiguousarray(wu),
              "wd": np.ascontiguousarray(wd),
              "wdh": np.ascontiguousarray(wdh)}
        for e in range(E):
            im[f"xt{e}"] = xts[e]
            im[f"wr{e}"] = wrs[e]
        in_maps.append(im)

    trace = bool(int(os.environ.get("KERNEL_TRACE", "0")))
    old_m = nc.m
    nc.m = get_hw_module(nc.m)
    try:
        try:
            res = run_bass_kernel_spmd(nc, in_maps, core_ids=list(range(E)),
                                       trace=trace)
        except (ImportError, ModuleNotFoundError):
            os.environ["BASS_NEVER_TRACE"] = "1"
            res = run_bass_kernel_spmd(nc, in_maps, core_ids=list(range(E)),
                                       trace=False)
    finally:
        nc.m = old_m
    LAST_RESULT = res

    # ---- combine: sum per-core F-slice partials, scatter into out ----
    out = np.zeros((T, D), dtype=np.float32)
    for e in range(E):
        n = ns[e]
        acc = res.results[0][f"out{e}"].astype(np.float64)
        for c in range(1, E):
            acc += res.results[c][f"out{e}"]
        out[idx[e]] += acc.reshape(D, ms[e])[:, :n].T.astype(np.float32)
    return out


# revision 6
# speedup vs baseline: 1.0923x; 1.0129x over previous
"""BlockSparseMLP (MoE top-2, 8 experts) — Trainium2 Bass kernel, balanced.

Sharding: instead of expert-per-core (which pads every core to the most
popular expert's token count), each core processes ALL 8 experts but only
a 1/8 slice of the F dimension (704 of 5632 columns).  Per-core weight
bytes stay exactly one expert-equivalent (F-slicing is traffic-neutral),
while the compute is perfectly balanced: every core does sum(n_e) = 4096
token-equivalents with NO padding.  Per-core partial d outputs (each a
partial sum over its F-slice) are weighted by the combine weights
on-device and summed across cores on the host (the unshard step).

The 704-column slice is 5 full 128-col PE tiles plus a 64-col half tile.
Half tiles are processed at full PE efficiency by pairing experts
(2k, 2k+1):
  phase 1: two col-tiled MMs (tile_position (0,0)/(0,64)) into separate
           PSUM banks run concurrently in different column groups.
  phase 2: the packed activations [ah_e2k ; ah_e2k1] feed two row-tiled
           K=64 MMs (stationary at partitions 0:64 / 64:128) that
           accumulate onto each expert's full-K d psum.

Everything bf16 on the wire and in the PE (fp32 PSUM), host pre-casts.
"""

import os
from itertools import zip_longest

import numpy as np
import ml_dtypes

T, D, F, E, TOPK = 2048, 2048, 5632, 8, 2
P = 128
KD = D // P       # 16 contraction subtiles over D
FS = F // E       # 704 F-columns per core
NFT = FS // P     # 5 full f-tiles
HOFF = NFT * P    # 640: half-tile column offset inside the slice
NPAIR = E // 2

BF16 = ml_dtypes.bfloat16

_COMPILED = {}     # tuple(m_e) -> nc
LAST_RESULT = None


def _to_bf16(a):
    """fp32 ndarray -> bf16 (RNE), vectorized."""
    u = np.ascontiguousarray(a, dtype=np.float32).view(np.uint32)
    r = ((u + np.uint32(0x7FFF) + ((u >> np.uint32(16)) & np.uint32(1)))
         >> np.uint32(16)).astype(np.uint16)
    return r.view(BF16)


def _chunks(m):
    """Token chunks <= 512 (PSUM bank limit), each even."""
    if m <= 512:
        return [m]
    h = ((m // 2 + 1) // 2) * 2
    return [h, m - h]


def _build(ms):
    """Build + compile the SPMD Tile program for per-expert token counts
    ``ms`` (tuple of 8 even ints)."""
    import concourse.bass as bass  # noqa: F401
    import concourse.mybir as mybir
    import concourse.tile as tile
    from concourse import bacc

    f32 = mybir.dt.float32
    bf16 = mybir.dt.bfloat16
    mult = mybir.AluOpType.mult
    silu = mybir.ActivationFunctionType.Silu

    mpad = max(ms)
    echunks = [[(sum(_chunks(m)[:i]), c) for i, c in enumerate(_chunks(m))]
               for m in ms]

    nc = bacc.Bacc("TRN2", target_bir_lowering=False, debug=False,
                   enable_asserts=False, num_devices=E)

    wg_d = nc.dram_tensor("wg", [E, P, KD, FS], bf16,
                          kind="ExternalInput").ap()
    wu_d = nc.dram_tensor("wu", [E, P, KD, FS], bf16,
                          kind="ExternalInput").ap()
    wd_d = nc.dram_tensor("wd", [E, P, NFT, D], bf16,
                          kind="ExternalInput").ap()
    wdh_d = nc.dram_tensor("wdh", [NPAIR, P, D], bf16,
                           kind="ExternalInput").ap()
    xt_d = [nc.dram_tensor(f"xt{e}", [P, KD, ms[e]], bf16,
                           kind="ExternalInput").ap() for e in range(E)]
    wr_d = [nc.dram_tensor(f"wr{e}", [P, ms[e]], f32,
                           kind="ExternalInput").ap() for e in range(E)]
    out_d = [nc.dram_tensor(f"out{e}", [KD, P, ms[e]], bf16,
                            kind="ExternalOutput").ap() for e in range(E)]
    scr_d = nc.dram_tensor("scr", [P, 512], f32).ap()   # warm-up sink

    with tile.TileContext(nc) as tc:
        with (
            tc.tile_pool(name="resident", bufs=1) as rpool,
            tc.tile_pool(name="xtp", bufs=2) as xtpool,
            tc.tile_pool(name="w1", bufs=3) as w1pool,
            tc.tile_pool(name="whp", bufs=3) as whpool,
            tc.tile_pool(name="wdp", bufs=2) as wdpool,
            tc.tile_pool(name="wdhp", bufs=1) as wdhpool,
            tc.tile_pool(name="ap", bufs=3) as apool,
            tc.tile_pool(name="ahp", bufs=2) as ahpool,
            tc.tile_pool(name="wrp", bufs=4) as wrpool,
            tc.tile_pool(name="outp", bufs=4) as outpool,
            tc.tile_pool(name="psum", bufs=2, space="PSUM") as ppool,
        ):
            # Warm-up: open the PE HAM clock gate while first DMAs fly.
            warm = rpool.tile([P, 512], bf16)
            nc.vector.memset(warm[:], 0.0)
            wps = ppool.tile([P, 512], f32, tag="pg", name="warm_ps")
            NWARM = 28
            for i in range(NWARM):
                nc.tensor.matmul(wps[:], warm[:, :P], warm[:],
                                 start=(i == 0), stop=(i == NWARM - 1))
            wout = rpool.tile([P, P], f32)
            nc.vector.tensor_copy(out=wout[:], in_=wps[:, :P])
            nc.sync.dma_start(scr_d[:, :P], wout[:])

            xts, wgs, wus, whs, a_s, ahs, wrs = {}, {}, {}, {}, {}, {}, {}

            def load_expert(e, xt_first):
                """Issue input DMAs for expert e (order = arrival order).

                Main phase-1 weights are split 0:384 / 384:640 so each
                piece frees right after its full-tile MMs; the half-tile
                columns 640:704 live in small separate tiles that stay
                alive until the pair-halves pass (keeps the big tiles'
                buffer rotation from stalling the HWDGE ring).
                """
                m = ms[e]
                xt = xtpool.tile([P, KD, mpad], bf16, tag="xt",
                                 name=f"xt_{e}")
                wgh0 = w1pool.tile([P, KD, 384], bf16, tag="wg",
                                   name=f"wg_{e}_h0")
                wgh1 = w1pool.tile([P, KD, 384], bf16, tag="wg",
                                   name=f"wg_{e}_h1")
                wuh0 = w1pool.tile([P, KD, 384], bf16, tag="wu",
                                   name=f"wu_{e}_h0")
                wuh1 = w1pool.tile([P, KD, 384], bf16, tag="wu",
                                   name=f"wu_{e}_h1")
                wgh = whpool.tile([P, KD, 64], bf16, tag="wgh",
                                  name=f"wgh_{e}")
                wuh = whpool.tile([P, KD, 64], bf16, tag="wuh",
                                  name=f"wuh_{e}")
                wr = wrpool.tile([P, mpad], f32, tag="wr", name=f"wr_{e}")
                if xt_first:
                    nc.sync.dma_start(xt[:, :KD // 2, :m],
                                      xt_d[e][:, :KD // 2, :])
                    nc.sync.dma_start(wgh0[:], wg_d[e][:, :, 0:384])
                    nc.sync.dma_start(xt[:, KD // 2:, :m],
                                      xt_d[e][:, KD // 2:, :])
                    nc.sync.dma_start(wuh0[:], wu_d[e][:, :, 0:384])
                    nc.sync.dma_start(wgh1[:, :, :256],
                                      wg_d[e][:, :, 384:640])
                    nc.sync.dma_start(wuh1[:, :, :256],
                                      wu_d[e][:, :, 384:640])
                else:
                    nc.sync.dma_start(wgh0[:], wg_d[e][:, :, 0:384])
                    nc.sync.dma_start(wuh0[:], wu_d[e][:, :, 0:384])
                    nc.sync.dma_start(wgh1[:, :, :256],
                                      wg_d[e][:, :, 384:640])
                    nc.sync.dma_start(wuh1[:, :, :256],
                                      wu_d[e][:, :, 384:640])
                    nc.sync.dma_start(xt[:, :KD // 2, :m],
                                      xt_d[e][:, :KD // 2, :])
                    nc.sync.dma_start(xt[:, KD // 2:, :m],
                                      xt_d[e][:, KD // 2:, :])
                nc.sync.dma_start(wgh[:], wg_d[e][:, :, HOFF:FS])
                nc.sync.dma_start(wuh[:], wu_d[e][:, :, HOFF:FS])
                nc.gpsimd.dma_start(wr[:, :m], wr_d[e])
                xts[e] = xt
                wgs[e] = (wgh0, wgh1)
                wus[e] = (wuh0, wuh1)
                whs[e] = (wgh, wuh)
                wrs[e] = wr

            def w1sl(halves, ko, f0, f1):
                """Slice [f0:f1) of the 0:640 split phase-1 weight tiles."""
                if f1 <= 384:
                    return halves[0][:, ko, f0:f1]
                assert f0 >= 384
                return halves[1][:, ko, f0 - 384:f1 - 384]

            def phase1_full(e):
                m = ms[e]
                at = apool.tile([P, NFT, mpad], bf16, tag="a", name=f"a_{e}")
                a_s[e] = at
                for ft in range(NFT):
                    for (c0, cn) in echunks[e]:
                        pg = ppool.tile([P, 512], f32, tag="pg")
                        pu = ppool.tile([P, 512], f32, tag="pu")
                        for ko in range(KD):
                            nc.tensor.matmul(
                                pg[:, :cn],
                                w1sl(wgs[e], ko, ft * P, (ft + 1) * P),
                                xts[e][:, ko, c0:c0 + cn],
                                start=(ko == 0), stop=(ko == KD - 1))
                        for ko in range(KD):
                            nc.tensor.matmul(
                                pu[:, :cn],
                                w1sl(wus[e], ko, ft * P, (ft + 1) * P),
                                xts[e][:, ko, c0:c0 + cn],
                                start=(ko == 0), stop=(ko == KD - 1))
                        a_sl = at[:, ft, c0:c0 + cn]
                        nc.scalar.activation(a_sl, pg[:, :cn], silu)
                        nc.vector.tensor_tensor(a_sl, a_sl, pu[:, :cn], mult)

            def phase1_halves(pr):
                """Half f-tile (cols 640:704) for pair (2pr, 2pr+1):
                col-tiled concurrent MMs, separate PSUM banks."""
                e0, e1 = 2 * pr, 2 * pr + 1
                ah = ahpool.tile([P, mpad], bf16, tag="ah", name=f"ah_{pr}")
                ahs[pr] = ah
                for ca, cb in zip_longest(echunks[e0], echunks[e1]):
                    todo = []
                    if ca is not None:
                        todo.append((e0, 0, ca))
                    if cb is not None:
                        todo.append((e1, 64, cb))
                    pgs = {e: ppool.tile([P, 512], f32, tag="pg",
                                         name=f"pgh_{e}_{c0}")
                           for e, lo, (c0, cn) in todo}
                    pus = {e: ppool.tile([P, 512], f32, tag="pu",
                                         name=f"puh_{e}_{c0}")
                           for e, lo, (c0, cn) in todo}
                    for ko in range(KD):
                        for e, lo, (c0, cn) in todo:
                            nc.tensor.matmul(
                                pgs[e][lo:lo + 64, :cn],
                                whs[e][0][:, ko, :],
                                xts[e][:, ko, c0:c0 + cn],
                                start=(ko == 0), stop=(ko == KD - 1),
                                tile_position=(0, lo),
                                skip_group_check=True)
                    for ko in range(KD):
                        for e, lo, (c0, cn) in todo:
                            nc.tensor.matmul(
                                pus[e][lo:lo + 64, :cn],
                                whs[e][1][:, ko, :],
                                xts[e][:, ko, c0:c0 + cn],
                                start=(ko == 0), stop=(ko == KD - 1),
                                tile_position=(0, lo),
                                skip_group_check=True)
                    for e, lo, (c0, cn) in todo:
                        a_sl = ah[lo:lo + 64, c0:c0 + cn]
                        nc.scalar.activation(a_sl, pgs[e][lo:lo + 64, :cn],
                                             silu)
                        nc.vector.tensor_tensor(a_sl, a_sl,
                                                pus[e][lo:lo + 64, :cn], mult)

            def phase2(pr, wds, wdh):
                e0, e1 = 2 * pr, 2 * pr + 1
                for dt in range(KD):
                    dsl = slice(dt * P, (dt + 1) * P)
                    pds = {}
                    for e, lo, ptag in ((e0, 0, "pd0"), (e1, 64, "pd1")):
                        for ci, (c0, cn) in enumerate(echunks[e]):
                            pd = ppool.tile([P, 512], f32, tag=ptag,
                                            name=f"pd_{e}_{dt}_{ci}")
                            pds[(e, ci)] = pd
                            for fk in range(NFT):
                                nc.tensor.matmul(
                                    pd[:, :cn], wds[e][:, fk, dsl],
                                    a_s[e][:, fk, c0:c0 + cn],
                                    start=(fk == 0), stop=False)
                    # half contractions, row-tiled, emitted adjacently
                    for ca, cb in zip_longest(echunks[e0], echunks[e1]):
                        for e, lo, ch in ((e0, 0, ca), (e1, 64, cb)):
                            if ch is None:
                                continue
                            c0, cn = ch
                            ci = echunks[e].index(ch)
                            nc.tensor.matmul(
                                pds[(e, ci)][:, :cn],
                                wdh[lo:lo + 64, dsl],
                                ahs[pr][lo:lo + 64, c0:c0 + cn],
                                start=False, stop=True,
                                skip_group_check=True)
                    for e, lo, _pt in ((e0, 0, "pd0"), (e1, 64, "pd1")):
                        m = ms[e]
                        ot = outpool.tile([P, mpad], bf16, tag="ot",
                                          name=f"ot_{e}_{dt}")
                        for ci, (c0, cn) in enumerate(echunks[e]):
                            nc.vector.tensor_tensor(
                                ot[:, c0:c0 + cn], pds[(e, ci)][:, :cn],
                                wrs[e][:, c0:c0 + cn], mult)
                        nc.scalar.dma_start(out_d[e][dt][:, :m], ot[:, :m])

            load_expert(0, xt_first=True)
            load_expert(1, xt_first=True)
            for pr in range(NPAIR):
                e0, e1 = 2 * pr, 2 * pr + 1
                phase1_full(e0)
                if e1 + 1 < E:
                    load_expert(e1 + 1, xt_first=False)
                phase1_full(e1)
                # phase-2 weights ride the SWDGE (gpsimd) ring so their
                # buffer-rotation waits never block the phase-1 input
                # stream on the sync ring; emitted only here (a halves-
                # pass + a phase-1 of lead) so they don't steal HBM
                # bandwidth from the startup input stream
                wds = {}
                for e in (e0, e1):
                    wdt = wdpool.tile([P, NFT, D], bf16, tag="wd",
                                      name=f"wd_{e}")
                    nc.gpsimd.dma_start(wdt[:, :, 0:1024],
                                        wd_d[e][:, :, 0:1024])
                    nc.gpsimd.dma_start(wdt[:, :, 1024:D],
                                        wd_d[e][:, :, 1024:D])
                    wds[e] = wdt
                wdh = wdhpool.tile([P, D], bf16, tag="wdh", name=f"wdh_{pr}")
                nc.gpsimd.dma_start(wdh[:], wdh_d[pr])

                phase1_halves(pr)
                # after the halves, this pair's xt/half-weight tiles are
                # free — emitting the next load here keeps the HWDGE ring
                # from stalling on their buffer rotation mid-pair
                if e1 + 2 < E:
                    load_expert(e1 + 2, xt_first=False)
                phase2(pr, wds, wdh)
        # pools close

    nc.compile()
    return nc


def kernel(x, gate_tensor, Wg, Wu, Wd):
    global LAST_RESULT
    from concourse.bass_interp import get_hw_module
    from concourse.bass_utils import run_bass_kernel_spmd

    x = np.ascontiguousarray(np.asarray(x, dtype=np.float32))
    gate_tensor = np.asarray(gate_tensor, dtype=np.float32)

    # ---- router (replicated; tiny) ----
    logits = x @ gate_tensor
    mx = logits.max(axis=-1, keepdims=True)
    p = np.exp(logits - mx, dtype=np.float32)
    p /= p.sum(axis=-1, keepdims=True)
    topi = np.argsort(-p, axis=-1, kind="stable")[:, :TOPK]
    topw = np.take_along_axis(p, topi, axis=-1)
    topw = topw / (topw.sum(axis=-1, keepdims=True) + 1e-20)

    idx, wts = [], []
    for e in range(E):
        sel = (topi == e)
        idx.append(np.nonzero(sel.any(axis=-1))[0])
        wts.append(topw[sel].astype(np.float32))
    ns = [len(t) for t in idx]
    ms = tuple(max(2, n + (n & 1)) for n in ns)

    if ms not in _COMPILED:
        _COMPILED[ms] = _build(ms)
    nc = _COMPILED[ms]

    xb = _to_bf16(x)
    Wg_b = [_to_bf16(Wg[e]) for e in range(E)]
    Wu_b = [_to_bf16(Wu[e]) for e in range(E)]
    Wd_b = [_to_bf16(Wd[e]) for e in range(E)]

    # shared per-expert token tensors (same arrays for every core)
    xts, wrs = [], []
    for e in range(E):
        n, m = ns[e], ms[e]
        xt = np.zeros((P, KD, m), dtype=BF16)
        xt[:, :, :n] = xb[idx[e]].T.reshape(KD, P, n).transpose(1, 0, 2)
        wr = np.zeros((P, m), dtype=np.float32)
        wr[:, :n] = wts[e][None, :]
        xts.append(xt)
        wrs.append(wr)

    in_maps = []
    for c in range(E):
        c0 = c * FS
        wg = np.stack([Wg_b[e][:, c0:c0 + FS].reshape(KD, P, FS)
                       .transpose(1, 0, 2) for e in range(E)])
        wu = np.stack([Wu_b[e][:, c0:c0 + FS].reshape(KD, P, FS)
                       .transpose(1, 0, 2) for e in range(E)])
        wd = np.stack([Wd_b[e][c0:c0 + HOFF].reshape(NFT, P, D)
                       .transpose(1, 0, 2) for e in range(E)])
        wdh = np.stack([np.concatenate(
            [Wd_b[2 * pr][c0 + HOFF:c0 + FS],
             Wd_b[2 * pr + 1][c0 + HOFF:c0 + FS]], axis=0)
            for pr in range(NPAIR)])
        im = {"wg": np.ascontiguousarray(wg),
              "wu": np.ascontiguousarray(wu),
              "wd": np.ascontiguousarray(wd),
              "wdh": np.ascontiguousarray(wdh)}
        for e in range(E):
            im[f"xt{e}"] = xts[e]
            im[f"wr{e}"] = wrs[e]
        in_maps.append(im)

    trace = bool(int(os.environ.get("KERNEL_TRACE", "0")))
    old_m = nc.m
    nc.m = get_hw_module(nc.m)
    try:
        try:
            res = run_bass_kernel_spmd(nc, in_maps, core_ids=list(range(E)),
                                       trace=trace)
        except (ImportError, ModuleNotFoundError):
            os.environ["BASS_NEVER_TRACE"] = "1"
            res = run_bass_kernel_spmd(nc, in_maps, core_ids=list(range(E)),
                                       trace=False)
    finally:
        nc.m = old_m
    LAST_RESULT = res

    # ---- combine: sum per-core F-slice partials, scatter into out ----
    out = np.zeros((T, D), dtype=np.float32)
    for e in range(E):
        n = ns[e]
        acc = res.results[0][f"out{e}"].astype(np.float32)
        for c in range(1, E):
            acc += res.results[c][f"out{e}"].astype(np.float32)
        out[idx[e]] += acc.reshape(D, ms[e])[:, :n].T.astype(np.float32)
    return out


# revision 7
# speedup vs baseline: 1.1404x; 1.0440x over previous
"""BlockSparseMLP (MoE top-2 routing, 8 experts) — Trainium2 Bass kernel.

v2: weights and activations pre-cast to bf16 on the HOST, so HBM traffic
is halved (69 MB/core of weights instead of 138 MB).  Everything else as
the baseline: expert-per-core, host router/dispatch/combine, device runs
the gated MLP with fp32 PSUM accumulation.
"""

import os

import numpy as np
import ml_dtypes

T, D, F, E, TOPK = 2048, 2048, 5632, 8, 2
P = 128
KD = D // P     # 16 k-subtiles over D
KF = F // P     # 44 k-subtiles over F
FG = 4          # f-tiles per phase-1 weight DMA block (512 F columns)
NFG = KF // FG  # 11 phase-1 blocks
DG = 2          # d-tiles per phase-2 psum group (256 D columns)
NDG = KD // DG  # 8 phase-2 d-groups
KO2 = 4         # f-subtiles per phase-2 weight DMA block
NFB = KF // KO2  # 11 phase-2 blocks per d-group

_COMPILED = {}   # CAP -> (nc, chunk list)
LAST_RESULT = None  # BassKernelResults of the most recent run (for test.py)

BF16 = ml_dtypes.bfloat16


def _to_bf16(a):
    """fp32 ndarray -> bf16 (RNE), vectorized."""
    u = np.ascontiguousarray(a, dtype=np.float32).view(np.uint32)
    r = ((u + np.uint32(0x7FFF) + ((u >> np.uint32(16)) & np.uint32(1)))
         >> np.uint32(16)).astype(np.uint16)
    return r.view(BF16)


def _token_chunks(cap):
    """Split cap into free-dim chunks, each in [256, 512]."""
    assert cap >= 512 and cap % 2 == 0
    n512, rem = divmod(cap, 512)
    if rem == 0:
        return [512] * n512
    if rem >= 256:
        return [512] * n512 + [rem]
    return [512] * (n512 - 1) + [256 + rem, 256]


def _build(cap):
    """Build + compile the SPMD Tile program for token capacity `cap`."""
    import concourse.bass as bass  # noqa: F401
    import concourse.mybir as mybir
    import concourse.tile as tile
    from concourse import bacc

    f32 = mybir.dt.float32
    bf16 = mybir.dt.bfloat16
    mult = mybir.AluOpType.mult

    chunks = _token_chunks(cap)
    starts = [sum(chunks[:i]) for i in range(len(chunks))]

    nc = bacc.Bacc("TRN2", target_bir_lowering=False, debug=False,
                   enable_asserts=False, num_devices=E)

    xt_d = nc.dram_tensor("xt", [P, KD, cap], bf16, kind="ExternalInput").ap()
    wg_d = nc.dram_tensor("wg", [NFG, P, KD, P * FG], bf16,
                          kind="ExternalInput").ap()
    wu_d = nc.dram_tensor("wu", [NFG, P, KD, P * FG], bf16,
                          kind="ExternalInput").ap()
    wd_d = nc.dram_tensor("wd", [NDG, NFB, P, KO2, P * DG], bf16,
                          kind="ExternalInput").ap()
    wr_d = nc.dram_tensor("wrep", [P, cap], f32, kind="ExternalInput").ap()
    out_d = nc.dram_tensor("out_t", [D, cap], f32, kind="ExternalOutput").ap()
    scr_d = nc.dram_tensor("scr", [P, 512], f32).ap()   # warm-up sink

    with tile.TileContext(nc) as tc:
        with (
            tc.tile_pool(name="resident", bufs=1) as rpool,
            tc.tile_pool(name="w1", bufs=3) as w1pool,
            tc.tile_pool(name="wd2", bufs=8) as wd2pool,
            tc.tile_pool(name="outp", bufs=4) as outpool,
            tc.tile_pool(name="psum", bufs=2, space="PSUM") as ppool,
        ):
            xt = rpool.tile([P, KD, cap], bf16)
            wrep = rpool.tile([P, cap], f32)
            nc.sync.dma_start(wrep[:], wr_d)
            at = rpool.tile([P, KF, cap], bf16)

            # Warm-up: run throwaway matmuls while the first DMAs are in
            # flight so the PE HAM clock-gate opens (1.2 -> 2.4 GHz)
            # before real work arrives.
            warm = rpool.tile([P, 512], bf16)
            nc.vector.memset(warm[:], 0.0)
            wps = ppool.tile([P, 512], f32, tag="ps0c0", name="warm_ps")
            for i in range(20):
                nc.tensor.matmul(wps[:], warm[:, :P], warm[:],
                                 start=(i == 0), stop=(i == 19))
            wout = rpool.tile([P, 512], f32)
            nc.vector.tensor_copy(out=wout[:], in_=wps[:])
            nc.sync.dma_start(scr_d[:], wout[:])

            # Queue order on the single SWDGE ring decides arrival order:
            # first weight sub-block + first token slices (so PE can start
            # early), then the token bulk, then the stream.
            nc.gpsimd.dma_start(xt[:, :2, :], xt_d[:, :2, :])

            w1tiles = []
            for fg in range(NFG):
                wgb = w1pool.tile([P, KD, P * FG], bf16, tag="wgb",
                                  name=f"wgb_{fg}")
                wub = w1pool.tile([P, KD, P * FG], bf16, tag="wub",
                                  name=f"wub_{fg}")
                w1tiles.append((wgb, wub))
                if fg == 0:
                    # fine-grained first block + token bulk spread over
                    # several DMAs so multiple SWDGE lanes pull in parallel
                    for s in range(FG):
                        sl = slice(s * P, (s + 1) * P)
                        nc.gpsimd.dma_start(wgb[:, :, sl], wg_d[0][:, :, sl])
                        nc.gpsimd.dma_start(wub[:, :, sl], wu_d[0][:, :, sl])
                        if s == 0:
                            for k0 in range(2, KD, 2):
                                nc.gpsimd.dma_start(
                                    xt[:, k0:k0 + 2, :], xt_d[:, k0:k0 + 2, :])
                else:
                    kh = KD // 2
                    nc.gpsimd.dma_start(wgb[:, :kh, :], wg_d[fg][:, :kh, :])
                    nc.gpsimd.dma_start(wgb[:, kh:, :], wg_d[fg][:, kh:, :])
                    nc.gpsimd.dma_start(wub[:, :kh, :], wu_d[fg][:, :kh, :])
                    nc.gpsimd.dma_start(wub[:, kh:, :], wu_d[fg][:, kh:, :])

                # ---- phase 1: gT/uT = W.T @ xT, aT = silu(gT)*uT ----
                for fs in range(FG):
                    ft = fg * FG + fs
                    for ci, (c0, cn) in enumerate(zip(starts, chunks)):
                        pg = ppool.tile([P, cn], f32, tag=f"ps0c{ci}")
                        pu = ppool.tile([P, cn], f32, tag=f"ps1c{ci}")
                        for ko in range(KD):
                            nc.tensor.matmul(
                                pg[:], wgb[:, ko, fs * P:(fs + 1) * P],
                                xt[:, ko, c0:c0 + cn],
                                start=(ko == 0), stop=(ko == KD - 1))
                        for ko in range(KD):
                            nc.tensor.matmul(
                                pu[:], wub[:, ko, fs * P:(fs + 1) * P],
                                xt[:, ko, c0:c0 + cn],
                                start=(ko == 0), stop=(ko == KD - 1))
                        a_sl = at[:, ft, c0:c0 + cn]
                        nc.scalar.activation(
                            a_sl, pg[:], mybir.ActivationFunctionType.Silu)
                        nc.vector.tensor_tensor(a_sl, a_sl, pu[:], mult)

            # ---- phase 2: dT = Wd.T @ aT, out = dT * w ----
            for dg in range(NDG):
                pds = [[ppool.tile([P, cn], f32, tag=f"ps{ds}c{ci}",
                                   name=f"pd_{dg}_{ds}_{ci}")
                        for ci, cn in enumerate(chunks)]
                       for ds in range(DG)]
                for fb in range(NFB):
                    wdb = wd2pool.tile([P, KO2, P * DG], bf16, tag="wdb")
                    nc.gpsimd.dma_start(wdb[:], wd_d[dg, fb])
                    for ko in range(KO2):
                        fk = fb * KO2 + ko
                        for ds in range(DG):
                            for ci, (c0, cn) in enumerate(zip(starts, chunks)):
                                nc.tensor.matmul(
                                    pds[ds][ci][:],
                                    wdb[:, ko, ds * P:(ds + 1) * P],
                                    at[:, fk, c0:c0 + cn],
                                    start=(fk == 0), stop=(fk == KF - 1))
                for ds in range(DG):
                    ot = outpool.tile([P, cap], f32, tag="ot")
                    for ci, (c0, cn) in enumerate(zip(starts, chunks)):
                        nc.vector.tensor_tensor(
                            ot[:, c0:c0 + cn], pds[ds][ci][:],
                            wrep[:, c0:c0 + cn], mult)
                    dt_idx = dg * DG + ds
                    nc.sync.dma_start(
                        out_d[dt_idx * P:(dt_idx + 1) * P, :], ot[:])

    nc.compile()
    return nc, chunks


def _swizzle_w1(w):
    """bf16 [D, F] -> [NFG, P, KD, P*FG] block-major, partition-contiguous."""
    return np.ascontiguousarray(
        w.reshape(KD, P, NFG, P * FG).transpose(2, 1, 0, 3))


def _swizzle_wd(w):
    """bf16 [F, D] -> [NDG, NFB, P, KO2, P*DG] block-major."""
    return np.ascontiguousarray(
        w.reshape(NFB, KO2, P, NDG, P * DG).transpose(3, 0, 2, 1, 4))


def kernel(x, gate_tensor, Wg, Wu, Wd):
    global LAST_RESULT
    from concourse.bass_interp import get_hw_module
    from concourse.bass_utils import run_bass_kernel_spmd

    x = np.ascontiguousarray(np.asarray(x, dtype=np.float32))
    gate_tensor = np.asarray(gate_tensor, dtype=np.float32)

    # ---- router (replicated; tiny: T*D*E flops) ----
    logits = x @ gate_tensor                      # [T, E] fp32
    m = logits.max(axis=-1, keepdims=True)
    p = np.exp(logits - m, dtype=np.float32)
    p /= p.sum(axis=-1, keepdims=True)
    topi = np.argsort(-p, axis=-1, kind="stable")[:, :TOPK]      # [T, K]
    topw = np.take_along_axis(p, topi, axis=-1)
    topw = topw / (topw.sum(axis=-1, keepdims=True) + 1e-20)

    idx = []          # tokens routed to each expert
    wts = []          # their combine weights
    for e in range(E):
        sel = (topi == e)                         # [T, K]; <=1 True per row
        idx.append(np.nonzero(sel.any(axis=-1))[0])
        wts.append(topw[sel].astype(np.float32))  # row-major == token order
    max_n = max(len(t) for t in idx)
    cap = max(512, ((max_n + 1) // 2) * 2)

    if cap not in _COMPILED:
        _COMPILED[cap] = _build(cap)
    nc, _chunks = _COMPILED[cap]

    xb = _to_bf16(x)                              # [T, D] bf16

    # ---- dispatch: per-core inputs (pre-swizzled to SBUF block layout) ----
    in_maps = []
    for e in range(E):
        n = len(idx[e])
        xg = xb[idx[e]]                           # [n, D] bf16
        xt = np.zeros((P, KD, cap), dtype=BF16)
        xt[:, :, :n] = xg.T.reshape(KD, P, n).transpose(1, 0, 2)
        wr = np.zeros((P, cap), dtype=np.float32)
        wr[:, :n] = wts[e][None, :]
        in_maps.append({"xt": xt,
                        "wg": _swizzle_w1(_to_bf16(Wg[e])),
                        "wu": _swizzle_w1(_to_bf16(Wu[e])),
                        "wd": _swizzle_wd(_to_bf16(Wd[e])),
                        "wrep": wr})

    trace = bool(int(os.environ.get("KERNEL_TRACE", "0")))
    old_m = nc.m
    nc.m = get_hw_module(nc.m)
    try:
        try:
            res = run_bass_kernel_spmd(nc, in_maps, core_ids=list(range(E)),
                                       trace=trace)
        except (ImportError, ModuleNotFoundError):
            os.environ["BASS_NEVER_TRACE"] = "1"
            res = run_bass_kernel_spmd(nc, in_maps, core_ids=list(range(E)),
                                       trace=False)
    finally:
        nc.m = old_m
    LAST_RESULT = res

    # ---- combine: scatter-add the per-expert partials ----
    out = np.zeros((T, D), dtype=np.float32)
    for e in range(E):
        n = len(idx[e])
        out[idx[e]] += res.results[e]["out_t"][:, :n].T
    return out
